# revision 20
# baseline (speedup 1.0000x reference)
"""Trainium2 Bass kernel for a dense pre-norm transformer block (fp8 version).

Problem: x[8, 1024, 768]; per-batch-element transformer block
  (LN1 -> qkv -> 12-head attention -> proj residual -> LN2 -> MLP(gelu) residual).

Strategy (v2, fp8 DoubleRow):
  - Pure data-parallel: 8 NeuronCores, one batch element each. No collectives.
  - Activations channel-major ("T layout", [C, tokens]); host transposes.
  - Large GEMMs (qkv, v, attn@V, proj, fc1, fc2) run as fp8e4m3 DoubleRow
    matmuls: both operands fp8, two 128-deep k-planes per instruction at
    ~2x the bf16 column rate (measured ~2.6x incl. overheads).
  - Scores stay bf16 with two heads row-split across the PE array
    (tile_position (0,0)/(64,0)) which overlap nearly fully.
  - Scale management: residual stream xsb holds S*x (S=256, f32). LN is
    scale-invariant (eps scaled by S^2), producing unit-scale z in fp8; the
    LN affine (g,b) is folded into the following GEMM's weights/bias on the
    host. Weights carry TW=256 (qkv/fc1/fc2) or TWP=4 (proj); V activations
    carry TV=16, attention output TO=64; TO*TWP = TW = S makes every
    residual eviction a single fused scalar_tensor_tensor op.
  - exp emits fp8 directly (ScalarE); softmax denominators ride in 32 ones
    columns of the V tiles (stationary widths must be multiples of 32),
    broadcast back per head-pair via a DRAM round trip.
  - Engine balance: ScalarE = exp/gelu/sqrt only; DVE = almost all other
    elementwise (gpsimd/Pool ALU is 4-24x slower than DVE on HW, so Pool only
    issues software-DGE DMAs and the LN centered products).
"""

import ml_dtypes
import numpy as np

import concourse.bacc as bacc
import concourse.bass as bass
import concourse.mybir as mybir
from concourse import tile
from concourse.bass_utils import run_bass_kernel_spmd

AF = mybir.ActivationFunctionType
ALU = mybir.AluOpType
DR = mybir.MatmulPerfMode.DoubleRow
f32 = mybir.dt.float32
f32r = mybir.dt.float32r
bf16 = mybir.dt.bfloat16
fp8 = mybir.dt.float8e4

P = 128
DIM = 768
CT = DIM // P            # 6 channel tiles
NP = CT // 2             # 3 channel-tile pairs (DoubleRow)
N = 1024                 # tokens
NT = N // P              # 8 token tiles
NH = 12                  # heads
DH = 64                  # head dim
VC = 96                  # V tile cols: 64 dims + 32 ones (denominator) cols
HID = 3072
HT = HID // P            # 24 hidden tiles
HP = HT // 2             # 12 hidden-tile pairs
B = 8
EPS = 1e-5
SCALE = DH ** -0.5

S = 256.0                # residual-stream scale
TW = 256.0               # qkv/fc1/fc2 weight scale
TWP = 4.0                # proj weight scale
TO = 64.0                # attention-output scale
TV = 16.0                # V scale

E4 = ml_dtypes.float8_e4m3


def _t6(dram_2d):
    """View a [6*128, M] DRAM tensor/AP as [128, 6, M] (partition-major tiles)."""
    return dram_2d.rearrange("(a p) m -> p a m", p=P)


DEBUG_TAPS = False


def build_nc(reps=1):
    nc = bacc.Bacc("TRN2", target_bir_lowering=False, debug=False)

    # ---- I/O ----
    xT = nc.dram_tensor("xT", [DIM, N], f32, kind="ExternalInput")  # S*x^T
    wqk = nc.dram_tensor("wqk", [12, P, (CT + 2) * P], fp8, kind="ExternalInput")
    wv = nc.dram_tensor("wv", [P, CT + 2, DIM], fp8, kind="ExternalInput")
    wproj = nc.dram_tensor("wproj", [P, CT, DIM], fp8, kind="ExternalInput")
    wfc1 = nc.dram_tensor("wfc1", [HT, P, (CT + 2) * P], fp8, kind="ExternalInput")
    wfc2 = nc.dram_tensor("wfc2", [HP, 2, P, 2 * 3 * P], fp8, kind="ExternalInput")
    bqk = nc.dram_tensor("bqk", [P, 12], f32, kind="ExternalInput")
    bv = nc.dram_tensor("bv", [DIM], f32, kind="ExternalInput")      # TV * bv'
    bproj = nc.dram_tensor("bproj", [P, CT], f32, kind="ExternalInput")  # S * b
    bfc1 = nc.dram_tensor("bfc1", [P, HT], f32, kind="ExternalInput")
    bfc2 = nc.dram_tensor("bfc2", [P, CT], f32, kind="ExternalInput")   # S * b
    outT = nc.dram_tensor("outT", [DIM, N], f32, kind="ExternalOutput")

    taps = {}
    if DEBUG_TAPS:
        for name, shape, dt_ in [
            ("t_h1", [DIM, N], fp8), ("t_v", [P, NT, NH, VC], fp8),
            ("t_qt", [P, N], bf16), ("t_kt", [P, N], bf16),
            ("t_e0", [P, 2, N], fp8), ("t_o", [DIM, N], fp8),
            ("t_x2", [DIM, N], f32), ("t_h3", [P, HT, N], fp8),
            ("t_av", [VC, N], f32), ("t_rt", [P, N], bf16),
        ]:
            taps[name] = nc.dram_tensor(name, shape, dt_, kind="ExternalOutput")

    args = locals()
    with tile.TileContext(nc) as tc:
        _body(nc, tc, args, reps)
    nc.compile()
    return nc


def _body(nc, tc, t, reps=1):
    xT, outT = t["xT"], t["outT"]
    _dma_rr = [0]

    def dma_load(out, in_):
        nc.sync.dma_start(out, in_)

    wqk, wv, wproj, wfc1, wfc2 = t["wqk"], t["wv"], t["wproj"], t["wfc1"], t["wfc2"]
    taps = t["taps"]

    with (
        tc.tile_pool(name="const", bufs=1) as const,
        tc.tile_pool(name="resid", bufs=1) as resid,
        tc.tile_pool(name="hpool", bufs=1) as hpool,
        tc.tile_pool(name="dram", bufs=1, space="DRAM") as dram,
    ):
        # ---- residual stream (channel-major, f32, scaled by S) ----
        xsb = resid.tile([P, CT, N], f32)
        for ct in range(CT):
            dma_load(xsb[:, ct, :], xT[ct * P:(ct + 1) * P, :])

        # ---- constants ----
        ones2 = const.tile([P, 2 * P], fp8)
        nc.vector.memset(ones2[:], 1.0)
        ones2v = ones2[:].rearrange("p (s m) -> p s m", s=2)
        eps_t = const.tile([P, 1], f32)
        nc.vector.memset(eps_t[:], EPS * S * S)
        bqk_sb = const.tile([P, 12], f32)
        nc.sync.dma_start(bqk_sb[:], t["bqk"][:])
        bproj_sb = const.tile([P, CT], f32)
        nc.sync.dma_start(bproj_sb[:], t["bproj"][:])
        bfc1_sb = const.tile([P, HT], f32)
        nc.sync.dma_start(bfc1_sb[:], t["bfc1"][:])
        bfc2_sb = const.tile([P, CT], f32)
        nc.sync.dma_start(bfc2_sb[:], t["bfc2"][:])
        # v-bias (pre-scaled by TV) broadcast to all partitions
        vb_sb = const.tile([P, DIM], f32)
        bv_ap = t["bv"][:]
        bv_bcast = bass.AP(tensor=bv_ap.tensor, offset=bv_ap.offset,
                           ap=[[0, P], [1, DIM]])
        nc.gpsimd.dma_start(vb_sb[:], bv_bcast)

        def layer_norm_T(src, dst):
            """src: [P, CT, N] f32 (S-scaled); dst: [P, CT, N] fp8 = z (unit).

            Stats via fp8 DoubleRow ones-matmuls: xq = src/8, sq = src^2/16384
            (fp8e4 on TRN2 is IEEE e4m3: max finite 240); the stats noise is
            negligible relative to fp8 z quantization downstream.
            """
            with (
                tc.tile_pool(name="ln_tmp", bufs=1) as tmp,
                tc.tile_pool(name="ln_ps", bufs=1, space="PSUM") as lps,
            ):
                xq = tmp.tile([P, CT, N], fp8, name="xq")
                sq = tmp.tile([P, CT, N], fp8, name="sq")
                mu_ps = lps.tile([P, N], f32)
                e2_ps = lps.tile([P, N], f32)
                for ct in range(CT):
                    nc.vector.tensor_scalar(
                        out=xq[:, ct, :], in0=src[:, ct, :],
                        scalar1=0.125, scalar2=None, op0=ALU.mult)
                    nc.vector.scalar_tensor_tensor(
                        out=sq[:, ct, :], in0=src[:, ct, :],
                        scalar=1.0 / 16384, op0=ALU.mult,
                        in1=src[:, ct, :], op1=ALU.mult)
                for p in range(NP):
                    for h in range(2):
                        sl = bass.ts(h, 512)
                        nc.tensor.matmul(
                            mu_ps[:, sl], ones2v, xq[:, 2 * p:2 * p + 2, sl],
                            start=(p == 0), stop=(p == NP - 1), perf_mode=DR)
                        nc.tensor.matmul(
                            e2_ps[:, sl], ones2v, sq[:, 2 * p:2 * p + 2, sl],
                            start=(p == 0), stop=(p == NP - 1), perf_mode=DR)
                mu_sb = tmp.tile([P, N], f32)
                nc.vector.tensor_scalar(
                    out=mu_sb[:], in0=mu_ps[:], scalar1=8.0 / DIM,
                    scalar2=None, op0=ALU.mult)
                mu2 = tmp.tile([P, N], f32)
                nc.vector.tensor_mul(mu2[:], mu_sb[:], mu_sb[:])
                var = tmp.tile([P, N], f32)
                nc.vector.scalar_tensor_tensor(
                    out=var[:], in0=e2_ps[:], scalar=16384.0 / DIM,
                    op0=ALU.mult, in1=mu2[:], op1=ALU.subtract)
                sd = tmp.tile([P, N], f32)
                nc.scalar.activation(sd[:], var[:], AF.Sqrt, bias=eps_t[:],
                                     scale=1.0)
                rstd = tmp.tile([P, N], f32)
                nc.vector.reciprocal(rstd[:], sd[:])
                for s in range(2):
                    nc.vector.scalar_tensor_tensor(
                        out=dst[:, CT + s, :], in0=mu_sb[:], scalar=8.0,
                        op0=ALU.mult, in1=rstd[:], op1=ALU.mult)
                for ct in range(CT):
                    nc.gpsimd.tensor_mul(dst[:, ct, :], src[:, ct, :], rstd[:])

        for _rep in range(reps):
            with (
                tc.tile_pool(name="qkv_w", bufs=1) as qw,
                tc.tile_pool(name="attn", bufs=1) as attn,
                tc.tile_pool(name="pj_w", bufs=1) as pw,
            ):
                vsb = attn.tile([P, NT, NH, VC], fp8)
                osb = attn.tile([P, CT, N], fp8)
                wp_sb = pw.tile([P, CT, DIM], fp8)

                # ======== LN1 ========
                h1 = hpool.tile([P, CT + 2, N], fp8, tag="h")
                layer_norm_T(xsb, h1)
                nc.sync.dma_start(wp_sb[:], wproj[:])
                if taps:
                    nc.sync.dma_start(_t6(taps["t_h1"]), h1[:])

                dscr = dram.tile([NH, N], bf16, tag="dscr")
                # ==== merged QKV + attention, pipelined per head pair ====
                with (
                    tc.tile_pool(name="att_sb", bufs=1) as asb,
                    tc.tile_pool(name="att_ps", bufs=1, space="PSUM") as aps,
                ):
                    # ones columns (denominator lanes) of V tiles
                    nc.vector.memset(vsb[:, :, :, DH:VC], TV)
                    wv_sb = qw.tile([P, CT + 2, DIM], fp8)
                    nc.sync.dma_start(wv_sb[:], wv[:])
                    wv3 = wv_sb[:].rearrange("p (a s) m -> p a s m", s=2)

                    def v_tile(it):
                        vps = aps.tile([P, N], f32, tag="sc", bufs=2, name="vps")
                        for c0, cn in ((0, 512), (512, 256)):
                            for p in range(NP + 1):
                                nc.tensor.matmul(
                                    vps[:, c0:c0 + cn],
                                    h1[:, 2 * p:2 * p + 2, it * P:(it + 1) * P],
                                    wv3[:, p, :, c0:c0 + cn],
                                    start=(p == 0), stop=(p == NP),
                                    perf_mode=DR)
                        nc.vector.scalar_tensor_tensor(
                            out=vsb[:, it, :, 0:DH],
                            in0=vps[:, 0:DIM].rearrange("p (h d) -> p h d", d=DH),
                            scalar=TV / TW, op0=ALU.mult,
                            in1=vb_sb[:].rearrange("p (h d) -> p h d", d=DH),
                            op1=ALU.add)

                    def qk_prod(tp):
                        qt = asb.tile([P, N], bf16, tag="qt", bufs=2, name="qt")
                        kt2 = asb.tile([P, N], bf16, tag="kt2", bufs=2,
                                       name="kt2")
                        for dst_sb, mt in ((qt, tp), (kt2, CT + tp)):
                            wt = qw.tile([P, (CT + 2) * P], fp8, tag="wqk",
                                         bufs=2, name="wt")
                            dma_load(wt[:], wqk[mt, :, :])
                            wt3 = wt[:].rearrange("p (a s m) -> p a s m",
                                                  s=2, m=P)
                            qkps = aps.tile([P, N], f32, tag="sc", bufs=2,
                                            name="qkps")
                            for h in range(2):
                                sl = bass.ts(h, 512)
                                for p in range(NP + 1):
                                    nc.tensor.matmul(
                                        qkps[:, sl], wt3[:, p, :, :],
                                        h1[:, 2 * p:2 * p + 2, sl],
                                        start=(p == 0), stop=(p == NP),
                                        perf_mode=DR)
                            nc.vector.tensor_scalar(
                                out=dst_sb[:], in0=qkps[:],
                                scalar1=1.0 / TW,
                                scalar2=bqk_sb[:, mt:mt + 1],
                                op0=ALU.mult, op1=ALU.add)
                        return qt, kt2

                    def attn_jp(tp, jp, qt, kt2, av0, av1, with_v):
                        e0 = asb.tile([P, 2, N], fp8, tag="e0", bufs=2,
                                      name="e0")
                        e1 = asb.tile([P, 2, N], fp8, tag="e1", bufs=2,
                                      name="e1")
                        for s in range(2):
                            jt = 2 * jp + s
                            if with_v:
                                v_tile(jt)
                            sc0 = aps.tile([P, N], f32, tag="sc", bufs=2,
                                           name="sc0")
                            sc1 = aps.tile([P, N], f32, tag="sc", bufs=2,
                                           name="sc1")
                            js = slice(jt * P, (jt + 1) * P)
                            for h in range(2):
                                sl = bass.ts(h, 512)
                                nc.tensor.matmul(
                                    sc0[:, sl], kt2[0:DH, js],
                                    qt[0:DH, sl], tile_position=(0, 0))
                                nc.tensor.matmul(
                                    sc1[:, sl], kt2[DH:P, js],
                                    qt[DH:P, sl], tile_position=(DH, 0))
                            nc.scalar.activation(e0[:, s, :], sc0[:], AF.Exp,
                                                 scale=SCALE)
                            nc.scalar.activation(e1[:, s, :], sc1[:], AF.Exp,
                                                 scale=SCALE)
                        if taps and tp == 0 and jp == 0:
                            nc.sync.dma_start(taps["t_e0"][:], e0[:])
                        for h in range(2):
                            sl = bass.ts(h, 512)
                            nc.tensor.matmul(
                                av0[:, sl], vsb[:, 2 * jp:2 * jp + 2, 2 * tp, :],
                                e0[:, :, sl],
                                start=(jp == 0), stop=(jp == NT // 2 - 1),
                                perf_mode=DR)
                            nc.tensor.matmul(
                                av1[:, sl],
                                vsb[:, 2 * jp:2 * jp + 2, 2 * tp + 1, :],
                                e1[:, :, sl],
                                start=(jp == 0), stop=(jp == NT // 2 - 1),
                                perf_mode=DR)

                    def finish_pair(tp, av0, av1):
                        te = asb.tile([VC, N], bf16, tag="teto", bufs=4,
                                      name="te")
                        nc.vector.tensor_copy(te[DH:DH + 1, :],
                                              av0[DH:DH + 1, :])
                        nc.sync.dma_start(dscr[2 * tp, :], te[DH:DH + 1, :])
                        to = asb.tile([VC, N], bf16, tag="teto", bufs=4,
                                      name="to")
                        nc.vector.tensor_copy(to[0:DH + 1, :],
                                              av1[0:DH + 1, :])
                        nc.sync.dma_start(dscr[2 * tp + 1, :], to[DH:DH + 1, :])
                        # Rt rows 0:64 <- 1/denom0, rows 64:128 <- 1/denom1
                        rt = asb.tile([P, N], bf16, tag="rt", bufs=2, name="rt")
                        d_ap = dscr[:]
                        src = bass.AP(
                            tensor=d_ap.tensor,
                            offset=d_ap.offset + 2 * tp * N,
                            ap=[[N, 2], [0, DH], [1, N]])
                        nc.gpsimd.dma_start(rt[:], src)
                        with nc.allow_low_precision(reason="softmax denom"):
                            nc.vector.reciprocal(rt[:], rt[:])
                        if taps and tp == 0:
                            nc.sync.dma_start(taps["t_rt"][:], rt[:])
                        nc.vector.scalar_tensor_tensor(
                            out=osb[0:DH, tp, :], in0=av0[0:DH, :],
                            scalar=TO, op0=ALU.mult,
                            in1=rt[0:DH, :], op1=ALU.mult)
                        tos = asb.tile([P, N], bf16, tag="tos", bufs=2,
                                       name="tos")
                        nc.sync.dma_start(tos[DH:P, :], to[0:DH, :])
                        nc.vector.scalar_tensor_tensor(
                            out=osb[DH:P, tp, :], in0=tos[DH:P, :],
                            scalar=TO, op0=ALU.mult,
                            in1=rt[DH:P, :], op1=ALU.mult)

                    # pair 0 interleaved with V production
                    qt0, kt20 = qk_prod(0)
                    if taps:
                        nc.sync.dma_start(taps["t_qt"][:], qt0[:])
                        nc.sync.dma_start(taps["t_kt"][:], kt20[:])
                    av0 = aps.tile([VC, N], f32, tag="av", bufs=2, name="av0")
                    av1 = aps.tile([VC, N], f32, tag="av", bufs=2, name="av1")
                    for jp in range(NT // 2):
                        attn_jp(0, jp, qt0, kt20, av0, av1, with_v=True)
                    if taps:
                        tav = asb.tile([VC, N], f32, name="tav")
                        nc.vector.tensor_copy(tav[:], av0[:])
                        nc.sync.dma_start(taps["t_av"][:], tav[:])
                    finish_pair(0, av0, av1)
                    if taps:
                        nc.sync.dma_start(taps["t_v"][:], vsb[:])
                    for tp in range(1, CT):
                        qt, kt2 = qk_prod(tp)
                        av0 = aps.tile([VC, N], f32, tag="av", bufs=2,
                                       name="av0")
                        av1 = aps.tile([VC, N], f32, tag="av", bufs=2,
                                       name="av1")
                        for jp in range(NT // 2):
                            attn_jp(tp, jp, qt, kt2, av0, av1, with_v=False)
                        finish_pair(tp, av0, av1)
                if taps:
                    nc.sync.dma_start(_t6(taps["t_o"]), osb[:])

                # ======== proj + residual ========
                wp3 = wp_sb[:].rearrange("p (a s) m -> p a s m", s=2)
                with (
                    tc.tile_pool(name="pj_ps", bufs=1, space="PSUM") as pps,
                ):
                    for mt in range(CT):
                        for h in range(2):
                            sl = bass.ts(h, 512)
                            ps = pps.tile([P, 512], f32, tag="ps", bufs=6,
                                          name="ps")
                            for p in range(NP):
                                nc.tensor.matmul(
                                    ps[:],
                                    wp3[:, p, :, mt * P:(mt + 1) * P],
                                    osb[:, 2 * p:2 * p + 2, sl],
                                    start=(p == 0), stop=(p == NP - 1),
                                    perf_mode=DR)
                            nc.vector.scalar_tensor_tensor(
                                out=xsb[:, mt, sl], in0=ps[:],
                                scalar=bproj_sb[:, mt:mt + 1], op0=ALU.add,
                                in1=xsb[:, mt, sl], op1=ALU.add)

            if taps:
                nc.sync.dma_start(_t6(taps["t_x2"]), xsb[:])

            # ======== LN2 + MLP ========
            with tc.tile_pool(name="mlp_w", bufs=1) as mw:
                h2 = hpool.tile([P, CT + 2, N], fp8, tag="h")
                layer_norm_T(xsb, h2)
                h3sb = mw.tile([P, HT, N], fp8, name="h3sb")
                with tc.tile_pool(name="fc1_ps", bufs=1, space="PSUM") as f1p:
                    for ct in range(HT):
                        w1t = mw.tile([P, (CT + 2) * P], fp8, tag="w1t",
                                      bufs=3, name="w1t")
                        dma_load(w1t[:], wfc1[ct, :, :])
                        w13 = w1t[:].rearrange("p (a s m) -> p a s m",
                                               s=2, m=P)
                        h3ps = f1p.tile([P, N], f32, tag="h3ps", bufs=3,
                                        name="h3ps")
                        for h in range(2):
                            sl = bass.ts(h, 512)
                            for p in range(NP + 1):
                                nc.tensor.matmul(
                                    h3ps[:, sl], w13[:, p, :, :],
                                    h2[:, 2 * p:2 * p + 2, sl],
                                    start=(p == 0), stop=(p == NP),
                                    perf_mode=DR)
                        nc.scalar.activation(
                            h3sb[:, ct, :], h3ps[:], AF.Gelu,
                            bias=bfc1_sb[:, ct:ct + 1], scale=1.0 / TW)
                    if taps:
                        nc.sync.dma_start(taps["t_h3"][:], h3sb[:])
                with tc.tile_pool(name="fc2_ps", bufs=1, space="PSUM") as f2p:
                    for g in range(2):  # output groups: mt 0-2, 3-5
                        f2ps = [f2p.tile([P, N], f32, tag=f"f2_{i}", bufs=1,
                                         name=f"f2ps{i}") for i in range(3)]
                        for kp in range(HP):
                            w2t = mw.tile([P, 2 * 3 * P], fp8, tag="w2t",
                                          bufs=3, name="w2t")
                            dma_load(w2t[:], wfc2[kp, g, :, :])
                            w23 = w2t[:].rearrange("p (s m) -> p s m", s=2)
                            for i in range(3):
                                for h in range(2):
                                    sl = bass.ts(h, 512)
                                    nc.tensor.matmul(
                                        f2ps[i][:, sl],
                                        w23[:, :, i * P:(i + 1) * P],
                                        h3sb[:, 2 * kp:2 * kp + 2, sl],
                                        start=(kp == 0), stop=(kp == HP - 1),
                                        perf_mode=DR)
                        for i in range(3):
                            mt = g * 3 + i
                            nc.vector.scalar_tensor_tensor(
                                out=xsb[:, mt, :], in0=f2ps[i][:],
                                scalar=bfc2_sb[:, mt:mt + 1], op0=ALU.add,
                                in1=xsb[:, mt, :], op1=ALU.add)
                            if _rep == reps - 1:
                                ostg = mw.tile([P, N], f32, tag="ostg",
                                               bufs=2, name="ostg")
                                nc.vector.tensor_scalar(
                                    out=ostg[:], in0=xsb[:, mt, :],
                                    scalar1=1.0 / S, scalar2=None,
                                    op0=ALU.mult)
                                nc.sync.dma_start(
                                    _t6(outT)[:, mt, :], ostg[:])


_NC_CACHE = None


def _get_nc():
    global _NC_CACHE
    if _NC_CACHE is None:
        _NC_CACHE = build_nc()
    return _NC_CACHE


def _q8(a):
    return np.clip(np.ascontiguousarray(a), -240.0, 240.0).astype(E4)


def _prep_shared(qkv_w, qkv_b, proj_w, proj_b, fc1_w, fc1_b, fc2_w, fc2_b,
                 ln1_g, ln1_b, ln2_g, ln2_b):
    c = lambda a: np.ascontiguousarray(np.asarray(a, dtype=np.float32))
    qkv_w = np.asarray(qkv_w, np.float32)
    qkv_b = np.asarray(qkv_b, np.float32)
    fc1_w = np.asarray(fc1_w, np.float32)
    fc1_b = np.asarray(fc1_b, np.float32)
    fc2_w = np.asarray(fc2_w, np.float32)
    proj_w = np.asarray(proj_w, np.float32)
    g1 = np.asarray(ln1_g, np.float32)
    b1 = np.asarray(ln1_b, np.float32)
    g2 = np.asarray(ln2_g, np.float32)
    b2 = np.asarray(ln2_b, np.float32)

    # fold LN affine into following GEMMs: h @ W + b == z @ (g*W) + (b1@W + b)
    wqk_f = g1[:, None] * qkv_w[:, :2 * DIM]          # [768, 1536]
    bqk_f = qkv_b[:2 * DIM] + b1 @ qkv_w[:, :2 * DIM]
    wv_f = g1[:, None] * qkv_w[:, 2 * DIM:]           # [768, 768]
    bv_f = qkv_b[2 * DIM:] + b1 @ qkv_w[:, 2 * DIM:]
    wfc1_f = g2[:, None] * fc1_w
    bfc1_f = fc1_b + b2 @ fc1_w

    # m2-correction planes: GEMMs consume h = [t1(6), 8*m2, 8*m2]; stationary
    # planes 6,7 hold -TW*colsum(W)/2048 so the pair contributes
    # -TW*colsum*m2 (i.e. the mean subtraction), making psum = TW * z @ W.
    def with_m2(warr, colsum):
        # warr: [..., P, CT, M] tiles layout; colsum: [..., M] (pre-fold TW)
        c4 = -TW * colsum / 2048.0
        extra = np.broadcast_to(c4[..., None, None, :],
                                (*c4.shape[:-1], P, 2, c4.shape[-1]))
        return np.concatenate([warr, extra], axis=-2)

    wqk_t = (TW * wqk_f).reshape(CT, P, 12, P).transpose(2, 1, 0, 3)
    wqk_cs = wqk_f.sum(0).reshape(12, P)
    wv_t = (TW * wv_f).reshape(CT, P, DIM).transpose(1, 0, 2)
    wfc1_t = (TW * wfc1_f).reshape(CT, P, HT, P).transpose(2, 1, 0, 3)
    wfc1_cs = wfc1_f.sum(0).reshape(HT, P)

    return {
        "wqk": _q8(with_m2(wqk_t, wqk_cs).reshape(12, P, (CT + 2) * P)),
        "wv": _q8(np.concatenate([
            wv_t,
            np.broadcast_to((-TW * wv_f.sum(0) / 2048.0)[None, None, :],
                            (P, 2, DIM))], axis=1)),
        "wproj": _q8((TWP * proj_w).reshape(CT, P, DIM).transpose(1, 0, 2)),
        "wfc1": _q8(with_m2(wfc1_t, wfc1_cs).reshape(HT, P, (CT + 2) * P)),
        "wfc2": _q8((TW * fc2_w).reshape(HP, 2, P, 2, 3 * P)
                    .transpose(0, 3, 2, 1, 4).reshape(HP, 2, P, 2 * 3 * P)),
        "bqk": c(bqk_f.reshape(12, P).T),
        "bv": c(TV * bv_f),
        "bproj": c(S * np.asarray(proj_b).reshape(CT, P).T),
        "bfc1": c(bfc1_f.reshape(HT, P).T),
        "bfc2": c(S * np.asarray(fc2_b).reshape(CT, P).T),
    }


def run(x, shared, **spmd_kwargs):
    nc = _get_nc()
    x = np.asarray(x, dtype=np.float32)
    in_maps = [
        {**shared, "xT": np.ascontiguousarray(S * x[b].T)} for b in range(B)
    ]
    res = run_bass_kernel_spmd(nc, in_maps, core_ids=list(range(B)), **spmd_kwargs)
    out = np.stack([res.results[b]["outT"].T for b in range(B)])
    return out.astype(np.float32), res


def kernel(x, ln1_g, ln1_b, qkv_w, qkv_b, proj_w, proj_b,
           ln2_g, ln2_b, fc1_w, fc1_b, fc2_w, fc2_b):
    shared = _prep_shared(qkv_w, qkv_b, proj_w, proj_b, fc1_w, fc1_b,
                          fc2_w, fc2_b, ln1_g, ln1_b, ln2_g, ln2_b)
    out, _ = run(x, shared)
    return out
